# revision 3
# baseline (speedup 1.0000x reference)
"""BertFusion cross-attention kernel for 8x Trainium2 NeuronCores.

Problem (per batch element b):
    scores = H_b @ Vh_b^T          # (L, V) = (2048, 1024)
    probs  = softmax(scores, -1)
    out_b  = probs @ Vh_b          # (L, D) = (2048, 1024)

Sharding: pure data-parallel over batch (B=8 == n_cores). Each core gets one
batch element. Host-side we pick matmul-friendly layouts while slicing:
  - ht: H_b pre-transposed + tiled  [16, 128, 1024]   ht[i,p,k*128+m] = H[i*128+m, k*128+p]
  - vt: Vh_b^T tiled                [8, 128, 1024]    vt[k,p,v]       = Vh[v, k*128+p]
  - vn: Vh_b natural tiled          [8, 128, 1024]    vn[j,p,d]       = Vh[j*128+p, d]

Device per core (flash-style over 16 row-tiles of 128 l-rows):
  mm1: S[l,v] (2 PSUM banks) = sum_k ht_chunk_k^T @ vt_chunk_k       (f32r, 1cyc/row)
  softmax along free axis v: DVE reduce_max -> ACT exp(bias=-max, accum row sums)
  PE-transposes P -> P^T (needed as the stationary operand of mm2)
  mm2: O[l,d] = sum_j ptT_chunk_j^T @ vn_chunk_j                      (f32r)
  ACT copy with per-row scale 1/sumexp, DMA out.
mm2 of row-tile i-1 is emitted between mm1 and the softmax of row-tile i so the
PE never waits on the softmax/transpose chain.
"""

import numpy as np

import concourse.bass as bass
import concourse.mybir as mybir
import concourse.tile as tile
from concourse.bass import ts
from concourse.bass_utils import run_bass_kernel_spmd
from concourse.masks import make_identity

# ---------------------------------------------------------------------------
# Workaround: the walrus build in this environment accepts only ONE sync-wait
# command per instruction, while Tile freely attaches several. Post-pass over
# the built module: for every instruction carrying more than one wait, hoist
# the extras onto standalone EventSemaphore carrier instructions inserted
# immediately before it on the same engine (identical blocking semantics:
# engine sequencers dispatch in order).
# ---------------------------------------------------------------------------
import bass_rust


def _split_multi_waits(nc, max_waits=1):
    for fn in nc.m.functions:
        for bb in fn.blocks:
            insts = bb.instructions
            need = any(
                i.sync_info is not None
                and i.sync_info.on_wait
                and len(i.sync_info.on_wait) > max_waits
                for i in insts
            )
            if not need:
                continue
            new = []
            for inst in insts:
                si = inst.sync_info
                if si is not None and si.on_wait and len(si.on_wait) > max_waits:
                    waits = list(si.on_wait)
                    extra, keep = waits[:-max_waits], waits[-max_waits:]
                    for w in extra:
                        c = mybir.InstEventSemaphore(name=f"I-esw-{nc.next_id()}")
                        c.engine = inst.engine
                        c.sync_info = bass_rust.SyncInfo(on_wait=[w], on_update=[])
                        new.append(c)
                    si.on_wait = keep
                    inst.sync_info = si
                new.append(inst)
            bb.instructions = new

# ---------------------------------------------------------------------------

B, L, V, D = 8, 2048, 1024, 1024
LT = 128                # l-rows per tile
NLT = L // LT           # 16 row tiles
KC = D // 128           # 8 contraction chunks (mm1)
JC = V // 128           # 8 contraction chunks (mm2)
F32 = mybir.dt.float32
N_CORES = 8


def build_nc(mm_dtype=mybir.dt.float32r, reps=1):
    """Build the single-core Bass module (SPMD across 8 cores)."""
    nc = bass.Bass("TRN2", target_bir_lowering=False, debug=False,
                   num_devices=N_CORES)
    # walrus requires f32r matmul operands to be *produced* as f32r, so the
    # matmul input tensors are declared with the matmul dtype end-to-end
    # (numpy view is float32 either way; bits pass through unchanged).
    mdt = mm_dtype
    ht = nc.dram_tensor("ht", [NLT, 128, D], mdt, kind="ExternalInput").ap()
    vt = nc.dram_tensor("vt", [KC, 128, V], mdt, kind="ExternalInput").ap()
    vn = nc.dram_tensor("vn", [JC, 128, D], mdt, kind="ExternalInput").ap()
    out = nc.dram_tensor("out", [NLT, 128, D], F32, kind="ExternalOutput").ap()

    Exp = mybir.ActivationFunctionType.Exp
    Copy = mybir.ActivationFunctionType.Copy
    X = mybir.AxisListType.X

    with tile.TileContext(nc) as tc:
        from contextlib import ExitStack
        with ExitStack() as st:
            cpool = st.enter_context(tc.tile_pool(name="const", bufs=1))
            vpool = st.enter_context(tc.tile_pool(name="vh", bufs=1))
            htp = st.enter_context(tc.tile_pool(name="htp", bufs=3))
            pp = st.enter_context(tc.tile_pool(name="pp", bufs=2))
            ptp = st.enter_context(tc.tile_pool(name="ptp", bufs=2))
            op = st.enter_context(tc.tile_pool(name="op", bufs=2))
            statp = st.enter_context(tc.tile_pool(name="statp", bufs=4))
            psS = st.enter_context(tc.tile_pool(name="psS", bufs=2, space="PSUM"))
            psPT = st.enter_context(tc.tile_pool(name="psPT", bufs=1, space="PSUM"))
            psO = st.enter_context(tc.tile_pool(name="psO", bufs=1, space="PSUM"))

            ident = cpool.tile([128, 128], F32, tag="ident")
            make_identity(nc, ident[:])

            vt_sb = []
            vn_sb = []
            for k in range(KC):
                t = vpool.tile([128, V], mdt, tag=f"vt{k}")
                nc.sync.dma_start(out=t[:], in_=vt[k])
                vt_sb.append(t)
            for j in range(JC):
                t = vpool.tile([128, D], mdt, tag=f"vn{j}")
                nc.sync.dma_start(out=t[:], in_=vn[j])
                vn_sb.append(t)

            def emit_mm2(state):
                ptt, rec, i = state
                o0 = psO.tile([128, 512], F32, tag="o0")
                o1 = psO.tile([128, 512], F32, tag="o1")
                for j in range(JC):
                    lhsT = ptt[:, ts(j, 128)]
                    nc.tensor.matmul(o0[:], lhsT, vn_sb[j][:, 0:512],
                                     start=(j == 0), stop=(j == JC - 1))
                    nc.tensor.matmul(o1[:], lhsT, vn_sb[j][:, 512:1024],
                                     start=(j == 0), stop=(j == JC - 1))
                ot = op.tile([128, D], F32, tag="o")
                nc.scalar.activation(ot[:, 0:512], o0[:], Copy, scale=rec[:])
                nc.scalar.activation(ot[:, 512:1024], o1[:], Copy, scale=rec[:])
                nc.sync.dma_start(out=out[i], in_=ot[:])

            for _ in range(reps):
                prev = None
                for i in range(NLT):
                    htt = htp.tile([128, D], mdt, tag="ht")
                    nc.sync.dma_start(out=htt[:], in_=ht[i])
                    s0 = psS.tile([128, 512], F32, tag="s0")
                    s1 = psS.tile([128, 512], F32, tag="s1")
                    for k in range(KC):
                        lhsT = htt[:, ts(k, 128)]
                        nc.tensor.matmul(s0[:], lhsT,
                                         vt_sb[k][:, 0:512],
                                         start=(k == 0), stop=(k == KC - 1))
                        nc.tensor.matmul(s1[:], lhsT,
                                         vt_sb[k][:, 512:1024],
                                         start=(k == 0), stop=(k == KC - 1))
                    # PE gap-filler: second matmul of the previous row tile.
                    if prev is not None:
                        emit_mm2(prev)

                    m0 = statp.tile([128, 1], F32, tag="m0")
                    m1 = statp.tile([128, 1], F32, tag="m1")
                    nc.vector.reduce_max(m0[:], s0[:], axis=X)
                    nc.vector.reduce_max(m1[:], s1[:], axis=X)
                    negmax = statp.tile([128, 1], F32, tag="negmax")
                    nc.vector.tensor_max(negmax[:], m0[:], m1[:])
                    nc.vector.tensor_scalar_mul(negmax[:], negmax[:], -1.0)

                    p = pp.tile([128, V], F32, tag="p")
                    es0 = statp.tile([128, 1], F32, tag="es0")
                    es1 = statp.tile([128, 1], F32, tag="es1")
                    nc.scalar.activation(p[:, 0:512], s0[:], Exp,
                                         bias=negmax[:], accum_out=es0[:])
                    nc.scalar.activation(p[:, 512:1024], s1[:], Exp,
                                         bias=negmax[:], accum_out=es1[:])
                    rec = statp.tile([128, 1], F32, tag="rec")
                    nc.vector.tensor_add(rec[:], es0[:], es1[:])
                    nc.vector.reciprocal(rec[:], rec[:])

                    ptps = psPT.tile([128, V], F32, tag="ptps")
                    for j in range(JC):
                        nc.tensor.transpose(ptps[:, ts(j, 128)], p[:, ts(j, 128)],
                                            ident[:])
                    ptt = ptp.tile([128, V], mdt, tag="pt")
                    nc.vector.tensor_copy(ptt[:, 0:512], ptps[:, 0:512])
                    nc.vector.tensor_copy(ptt[:, 512:1024], ptps[:, 512:1024])
                    prev = (ptt, rec, i)
                emit_mm2(prev)
    _split_multi_waits(nc)
    return nc


def _shard_inputs(hidden_states, visual_hidden_state):
    H = np.ascontiguousarray(np.asarray(hidden_states, dtype=np.float32))
    Vh = np.ascontiguousarray(np.asarray(visual_hidden_state, dtype=np.float32))
    in_maps = []
    for b in range(B):
        Hb = H[b]                       # (L, D)
        Vb = Vh[b]                      # (V, D)
        ht = np.ascontiguousarray(
            Hb.reshape(NLT, LT, KC, 128).transpose(0, 3, 2, 1)
        ).reshape(NLT, 128, D)
        vt = np.ascontiguousarray(Vb.reshape(V, KC, 128).transpose(1, 2, 0))
        vn = Vb.reshape(JC, 128, D)
        in_maps.append({"ht": ht, "vt": vt, "vn": vn})
    return in_maps


def kernel(hidden_states, visual_hidden_state):
    in_maps = _shard_inputs(hidden_states, visual_hidden_state)
    nc = build_nc()
    res = run_bass_kernel_spmd(nc, in_maps, list(range(N_CORES)))
    return np.stack([res.results[c]["out"].reshape(L, D) for c in range(N_CORES)])


if __name__ == "__main__":
    rng = np.random.default_rng(0)
    h = rng.standard_normal((B, L, D), dtype=np.float32)
    v = rng.standard_normal((B, V, D), dtype=np.float32)
    o = kernel(h, v)
    print("out", o.shape, o.dtype, o[0, 0, :4])


# revision 9
# speedup vs baseline: 609.2343x; 609.2343x over previous
"""BertFusion cross-attention kernel for 8x Trainium2 NeuronCores.

Problem (per batch element b):
    scores = H_b @ Vh_b^T          # (L, V) = (2048, 1024)
    probs  = softmax(scores, -1)
    out_b  = probs @ Vh_b          # (L, D) = (2048, 1024)

Sharding: pure data-parallel over batch (B=8 == n_cores). Each core gets one
batch element. Host-side we pick matmul-friendly layouts while slicing:
  - ht: H_b pre-transposed + tiled  [16, 128, 1024]   ht[i,p,k*128+m] = H[i*128+m, k*128+p]
  - vt: Vh_b^T tiled                [8, 128, 1024]    vt[k,p,v]       = Vh[v, k*128+p]
  - vn: Vh_b natural tiled          [8, 128, 1024]    vn[j,p,d]       = Vh[j*128+p, d]

Device per core (flash-style over 16 row-tiles of 128 l-rows):
  mm1: S[l,v] (2 PSUM banks) = sum_k ht_chunk_k^T @ vt_chunk_k       (f32r, 1cyc/row)
  softmax along free axis v: DVE reduce_max -> ACT exp(bias=-max, accum row sums)
  PE-transposes P -> P^T (needed as the stationary operand of mm2)
  mm2: O[l,d] = sum_j ptT_chunk_j^T @ vn_chunk_j                      (f32r)
  ACT copy with per-row scale 1/sumexp, DMA out.
mm2 of row-tile i-1 is emitted between mm1 and the softmax of row-tile i so the
PE never waits on the softmax/transpose chain.
"""

import numpy as np

import concourse.bass as bass
import concourse.mybir as mybir
import concourse.tile as tile
from concourse.bass import ts
from concourse.bass_utils import run_bass_kernel_spmd
from concourse.masks import make_identity

# ---------------------------------------------------------------------------
# Workaround: the walrus build in this environment accepts only ONE sync-wait
# command per instruction, while Tile freely attaches several. Post-pass over
# the built module: for every instruction carrying more than one wait, hoist
# the extras onto standalone EventSemaphore carrier instructions inserted
# immediately before it on the same engine (identical blocking semantics:
# engine sequencers dispatch in order).
# ---------------------------------------------------------------------------
import bass_rust
from concourse.tile import ScopedClock


def _dist_drain_and_barrier(self, tick_clock, wait_clock):
    """Kernel-tail drain with its sem waits spread across all five engines so
    they proceed in parallel (the following all-engine barrier restores the
    original semantics); the stock version serializes them on SP, and this
    walrus accepts only one wait per instruction anyway."""
    nc = self.nc
    drain_inst = nc.sync.drain()
    wait_clock.add_sem_waits(
        drain_inst.ins, ScopedClock({None: tick_clock.global_clock})
    )
    si = drain_inst.ins.sync_info
    if si is not None and si.on_wait and len(si.on_wait) > 1:
        waits = list(si.on_wait)
        si.on_wait = waits[:1]
        drain_inst.ins.sync_info = si
        engines = [
            mybir.EngineType.SP,
            mybir.EngineType.Activation,
            mybir.EngineType.DVE,
            mybir.EngineType.PE,
            mybir.EngineType.Pool,
        ]
        bb = nc.cur_bb.bb
        for n, w in enumerate(waits[1:]):
            c = mybir.InstEventSemaphore(name=f"I-esw-{nc.next_id()}")
            c.engine = engines[n % len(engines)]
            c.sync_info = bass_rust.SyncInfo(on_wait=[w], on_update=[])
            nc.register_instruction(c, overwrite=True)
            bb.add_instruction(c)

    nc.all_engine_barrier()
    assert self.sems is not None
    popped = nc._tile_sem_poison_stack.pop()
    assert popped is self._sem_poison
    nc.clear_and_free_semaphores(list(self.sems.allocated().values()))
    nc.all_engine_barrier()


tile.TileContext._drain_and_barrier = _dist_drain_and_barrier


def _split_multi_waits(nc, max_waits=1):
    for fn in nc.m.functions:
        for bb in fn.blocks:
            insts = bb.instructions
            need = any(
                i.sync_info is not None
                and i.sync_info.on_wait
                and len(i.sync_info.on_wait) > max_waits
                for i in insts
            )
            if not need:
                continue
            new = []
            for inst in insts:
                si = inst.sync_info
                if si is not None and si.on_wait and len(si.on_wait) > max_waits:
                    waits = list(si.on_wait)
                    extra, keep = waits[:-max_waits], waits[-max_waits:]
                    for w in extra:
                        c = mybir.InstEventSemaphore(name=f"I-esw-{nc.next_id()}")
                        c.engine = inst.engine
                        c.sync_info = bass_rust.SyncInfo(on_wait=[w], on_update=[])
                        new.append(c)
                    si.on_wait = keep
                    inst.sync_info = si
                new.append(inst)
            bb.instructions = new

# ---------------------------------------------------------------------------

B, L, V, D = 8, 2048, 1024, 1024
LT = 128                # l-rows per tile
NLT = L // LT           # 16 row tiles
KC = D // 128           # 8 contraction chunks (mm1)
JC = V // 128           # 8 contraction chunks (mm2)
F32 = mybir.dt.float32
N_CORES = 8


def build_nc(mm_dtype=mybir.dt.float32r, reps=1, loop_trips=0,
             loop_reload=True):
    """Build the single-core Bass module (SPMD across 8 cores)."""
    nc = bass.Bass("TRN2", target_bir_lowering=False, debug=False,
                   num_devices=N_CORES)
    # walrus requires f32r matmul operands to be *produced* as f32r, so the
    # matmul input tensors are declared with the matmul dtype end-to-end
    # (numpy view is float32 either way; bits pass through unchanged).
    mdt = mm_dtype
    ht = nc.dram_tensor("ht", [NLT, 128, D], mdt, kind="ExternalInput").ap()
    vt = nc.dram_tensor("vt", [KC, 128, V], mdt, kind="ExternalInput").ap()
    vn = nc.dram_tensor("vn", [JC, 128, D], mdt, kind="ExternalInput").ap()
    out = nc.dram_tensor("out", [NLT, 128, D], F32, kind="ExternalOutput").ap()

    Exp = mybir.ActivationFunctionType.Exp
    Copy = mybir.ActivationFunctionType.Copy
    X = mybir.AxisListType.X

    with tile.TileContext(nc) as tc:
        from contextlib import ExitStack
        with ExitStack() as st:
            cpool = st.enter_context(tc.tile_pool(name="const", bufs=1))
            vpool = st.enter_context(tc.tile_pool(name="vh", bufs=1))
            htp = st.enter_context(tc.tile_pool(name="htp", bufs=3))
            pp = st.enter_context(tc.tile_pool(name="pp", bufs=2))
            ptp = st.enter_context(tc.tile_pool(name="ptp", bufs=2))
            op = st.enter_context(tc.tile_pool(name="op", bufs=2))
            statp = st.enter_context(tc.tile_pool(name="statp", bufs=4))
            psS = st.enter_context(tc.tile_pool(name="psS", bufs=1, space="PSUM"))
            psPT = st.enter_context(tc.tile_pool(name="psPT", bufs=1, space="PSUM"))
            psO = st.enter_context(tc.tile_pool(name="psO", bufs=2, space="PSUM"))

            ident_f32 = cpool.tile([128, 128], F32, tag="ident_f32")
            make_identity(nc, ident_f32[:])
            ident = cpool.tile([128, 128], mdt, tag="ident")
            nc.vector.tensor_copy(ident[:], ident_f32[:])

            # DMA order = HBM bandwidth priority: vt chunks feed the very
            # first matmuls, the first two ht tiles come next, vn is only
            # needed ~10us in (first mm2).
            vt_sb = []
            vn_sb = []
            in_loop_reload = bool(loop_trips and loop_reload)
            for k in range(KC):
                t = vpool.tile([128, V], mdt, tag=f"vt{k}")
                if not in_loop_reload:
                    nc.sync.dma_start(out=t[:], in_=vt[k])
                vt_sb.append(t)

            def emit_mm2(state):
                ptt, rec, i = state
                o0 = psO.tile([128, 512], F32, tag="o0")
                o1 = psO.tile([128, 512], F32, tag="o1")
                for j in range(JC):
                    lhsT = ptt[:, ts(j, 128)]
                    nc.tensor.matmul(o0[:], lhsT, vn_sb[j][:, 0:512],
                                     start=(j == 0), stop=(j == JC - 1))
                    nc.tensor.matmul(o1[:], lhsT, vn_sb[j][:, 512:1024],
                                     start=(j == 0), stop=(j == JC - 1))
                ot = op.tile([128, D], F32, tag="o")
                nc.scalar.activation(ot[:, 0:512], o0[:], Copy, scale=rec[:])
                nc.scalar.activation(ot[:, 512:1024], o1[:], Copy, scale=rec[:])
                nc.sync.dma_start(out=out[i], in_=ot[:])

            def load_ht(i):
                htt = htp.tile([128, D], mdt, tag="ht")
                nc.sync.dma_start(out=htt[:], in_=ht[i])
                return htt

            first_rep = [True]

            def one_rep():
                prev = None
                first = first_rep[0]
                first_rep[0] = False
                if in_loop_reload:
                    # timing loop: pay the full vt/vn input DMA every trip
                    for k in range(KC):
                        nc.sync.dma_start(out=vt_sb[k][:], in_=vt[k])
                ht_tiles = [load_ht(0), load_ht(1)]
                if first:
                    for j in range(JC):
                        t = vpool.tile([128, D], mdt, tag=f"vn{j}")
                        nc.sync.dma_start(out=t[:], in_=vn[j])
                        vn_sb.append(t)
                elif in_loop_reload:
                    for j in range(JC):
                        nc.sync.dma_start(out=vn_sb[j][:], in_=vn[j])
                for i in range(NLT):
                    htt = ht_tiles[i]
                    if i + 2 < NLT:
                        ht_tiles.append(load_ht(i + 2))
                    s0 = psS.tile([128, 512], F32, tag="s0")
                    s1 = psS.tile([128, 512], F32, tag="s1")
                    for k in range(KC):
                        lhsT = htt[:, ts(k, 128)]
                        nc.tensor.matmul(s0[:], lhsT,
                                         vt_sb[k][:, 0:512],
                                         start=(k == 0), stop=(k == KC - 1))
                        nc.tensor.matmul(s1[:], lhsT,
                                         vt_sb[k][:, 512:1024],
                                         start=(k == 0), stop=(k == KC - 1))
                    # PE gap-filler: second matmul of the previous row tile.
                    if prev is not None:
                        emit_mm2(prev)

                    m0 = statp.tile([128, 1], F32, tag="m0")
                    m1 = statp.tile([128, 1], F32, tag="m1")
                    nc.vector.reduce_max(m0[:], s0[:], axis=X)
                    nc.vector.reduce_max(m1[:], s1[:], axis=X)
                    negmax = statp.tile([128, 1], F32, tag="negmax")
                    nc.vector.tensor_max(negmax[:], m0[:], m1[:])
                    nc.vector.tensor_scalar_mul(negmax[:], negmax[:], -1.0)

                    p = pp.tile([128, V], mdt, tag="p")
                    es0 = statp.tile([128, 1], F32, tag="es0")
                    es1 = statp.tile([128, 1], F32, tag="es1")
                    nc.scalar.activation(p[:, 0:512], s0[:], Exp,
                                         bias=negmax[:], accum_out=es0[:])
                    nc.scalar.activation(p[:, 512:1024], s1[:], Exp,
                                         bias=negmax[:], accum_out=es1[:])
                    rec = statp.tile([128, 1], F32, tag="rec")
                    nc.vector.tensor_add(rec[:], es0[:], es1[:])
                    nc.vector.reciprocal(rec[:], rec[:])

                    ptps = psPT.tile([128, V], mdt, tag="ptps")
                    for j in range(JC):
                        nc.tensor.transpose(ptps[:, ts(j, 128)], p[:, ts(j, 128)],
                                            ident[:])
                    ptt = ptp.tile([128, V], mdt, tag="pt")
                    nc.vector.tensor_copy(ptt[:, 0:512], ptps[:, 0:512])
                    nc.vector.tensor_copy(ptt[:, 512:1024], ptps[:, 512:1024])
                    prev = (ptt, rec, i)
                emit_mm2(prev)

            if loop_trips:
                with tc.For_i(0, loop_trips, 1):
                    one_rep()
            else:
                for _ in range(reps):
                    one_rep()
    _split_multi_waits(nc)
    return nc


def _shard_inputs(hidden_states, visual_hidden_state):
    H = np.ascontiguousarray(np.asarray(hidden_states, dtype=np.float32))
    Vh = np.ascontiguousarray(np.asarray(visual_hidden_state, dtype=np.float32))
    in_maps = []
    for b in range(B):
        Hb = H[b]                       # (L, D)
        Vb = Vh[b]                      # (V, D)
        ht = np.ascontiguousarray(
            Hb.reshape(NLT, LT, KC, 128).transpose(0, 3, 2, 1)
        ).reshape(NLT, 128, D)
        vt = np.ascontiguousarray(Vb.reshape(V, KC, 128).transpose(1, 2, 0))
        vn = Vb.reshape(JC, 128, D)
        in_maps.append({"ht": ht, "vt": vt, "vn": vn})
    return in_maps


def kernel(hidden_states, visual_hidden_state):
    in_maps = _shard_inputs(hidden_states, visual_hidden_state)
    nc = build_nc()
    res = run_bass_kernel_spmd(nc, in_maps, list(range(N_CORES)))
    return np.stack([res.results[c]["out"].reshape(L, D) for c in range(N_CORES)])


if __name__ == "__main__":
    rng = np.random.default_rng(0)
    h = rng.standard_normal((B, L, D), dtype=np.float32)
    v = rng.standard_normal((B, V, D), dtype=np.float32)
    o = kernel(h, v)
    print("out", o.shape, o.dtype, o[0, 0, :4])
